# revision 8
# baseline (speedup 1.0000x reference)
"""DA-RNN multitask kernel for Trainium2 (Bass/Tile), 8-core data-parallel.

Strategy:
- Shard batch B=2048 across 8 cores (256 rows/core, processed as 2 chunks of 128).
- On-chip layout is "transposed": state tensors are [feature_partition, batch_free].
- All gate nonlinearities are tanh (sigmoid(x) = 0.5 + 0.5*tanh(x/2), folded into
  host-prescaled weights), so one ACT table set (exp_and_others: tanh/exp/square/copy)
  serves the whole scan; a single switch to sqrt happens at the very end (LayerNorm).
- Cell state is kept scaled: S := 2c, H := 2h, which makes the whole LSTM cell
  3 fused scalar_tensor_tensor ops + per-half output ops on DVE.
- Encoder input-attention softmax runs in natural [batch, feature] layout (ACT exp
  with accum_out giving row sums); x_in is transposed back per step via PE transpose,
  carrying a ones row (bias) and the y_t row (decoder input) in the same transpose.
- LayerNorm gamma/beta are folded exactly into the head weights on the host.
"""

import numpy as np

B, T_FULL, F = 2048, 128, 32
HE = HD = 128
NCLS = 3
NCORES = 8
BC = B // NCORES          # 256 batch rows per core
G4 = 4 * HE               # 512 gate width

_cache = {}


def _build(T, close_idx, b2_nonzero, debug=False):
    import concourse.bacc as bacc
    import concourse.mybir as mybir
    from concourse.tile import TileContext

    F32 = mybir.dt.float32
    AF = mybir.ActivationFunctionType
    OP = mybir.AluOpType
    AX = mybir.AxisListType

    TF = T * F
    nc = bacc.Bacc("TRN2", target_bir_lowering=False)

    # ---------------- DRAM I/O ----------------
    Xd = nc.dram_tensor("x", [BC, TF], F32, kind="ExternalInput")
    ewhh_d = nc.dram_tensor("ewhh", [HE, G4], F32, kind="ExternalInput")
    ewihb_d = nc.dram_tensor("ewihb", [F + 1, G4], F32, kind="ExternalInput")
    aw1h_d = nc.dram_tensor("aw1h", [HE, HE], F32, kind="ExternalInput")
    aw1c_d = nc.dram_tensor("aw1c", [HE, HE], F32, kind="ExternalInput")
    ab1_d = nc.dram_tensor("ab1", [HE, 1], F32, kind="ExternalInput")
    aw2_d = nc.dram_tensor("aw2", [HE, F], F32, kind="ExternalInput")
    ab2_d = nc.dram_tensor("ab2", [1, F], F32, kind="ExternalInput")
    dwhh_d = nc.dram_tensor("dwhh", [HD, G4], F32, kind="ExternalInput")
    dwihb_d = nc.dram_tensor("dwihb", [2, G4], F32, kind="ExternalInput")
    fw1d_d = nc.dram_tensor("fw1d", [HD, HE], F32, kind="ExternalInput")
    fw1e_d = nc.dram_tensor("fw1e", [HE, HE], F32, kind="ExternalInput")
    fb1_d = nc.dram_tensor("fb1", [HE, 1], F32, kind="ExternalInput")
    fw2_d = nc.dram_tensor("fw2", [HE, 1], F32, kind="ExternalInput")
    hw5d_d = nc.dram_tensor("hw5d", [HD, 4], F32, kind="ExternalInput")
    hw5e_d = nc.dram_tensor("hw5e", [HE, 4], F32, kind="ExternalInput")
    csb_d = nc.dram_tensor("csrow", [1, 4], F32, kind="ExternalInput")
    bhb_d = nc.dram_tensor("bhrow", [1, 4], F32, kind="ExternalInput")
    idn_d = nc.dram_tensor("idn", [128, 128], F32, kind="ExternalInput")
    ecomb_d = nc.dram_tensor("ecomb", [64, 32 * 128], F32, kind="ExternalInput")
    outd = nc.dram_tensor("out", [BC, 4], F32, kind="ExternalOutput")
    if debug:
        dbg_encT = nc.dram_tensor("dbg_encT", [128, T * BC], F32, kind="ExternalOutput")
        dbg_D = nc.dram_tensor("dbg_D", [128, BC], F32, kind="ExternalOutput")
        dbg_SS = nc.dram_tensor("dbg_SS", [128, 2 * BC], F32, kind="ExternalOutput")
        dbg_t0 = nc.dram_tensor("dbg_t0", [128, BC], F32, kind="ExternalOutput")
        dbg_xT = nc.dram_tensor("dbg_xT", [F + 2, BC], F32, kind="ExternalOutput")
        dbg_th = nc.dram_tensor("dbg_th", [128, 8 * BC], F32, kind="ExternalOutput")
        dbg_sc = nc.dram_tensor("dbg_sc", [128, 2 * T], F32, kind="ExternalOutput")
        dbg_beta = nc.dram_tensor("dbg_beta", [128, 2 * T], F32, kind="ExternalOutput")
        dbg_ctx = nc.dram_tensor("dbg_ctx", [128, BC], F32, kind="ExternalOutput")
        dbg_cat = nc.dram_tensor("dbg_cat", [128, 512], F32, kind="ExternalOutput")

    with TileContext(nc) as tc:
        with tc.tile_pool(name="persist", bufs=1) as cp:
            # ------------- persistent tiles -------------
            encT = cp.tile([128, T * BC], F32)          # H history (=2h), [h, (t, b)]
            SS = cp.tile([128, 2 * BC], F32)            # [S_enc | S_dec] (=2c)
            HDt = cp.tile([128, 2 * BC], F32)           # [H_enc | D_dec] (=2h, 2d)
            Zro2 = cp.tile([128, 2 * BC], F32)          # zeros
            Zro = cp.tile([128, BC], F32)               # zeros
            idn = cp.tile([128, 128], F32)
            ones1 = cp.tile([1, 128], F32)
            fw1d = cp.tile([HD, HE], F32)
            fw1e = cp.tile([HE, HE], F32)
            fb1 = cp.tile([HE, 1], F32)
            fw2 = cp.tile([HE, 1], F32)
            hw5d = cp.tile([HD, 4], F32)
            hw5e = cp.tile([HE, 4], F32)
            csrow = cp.tile([1, 4], F32)
            bhrow = cp.tile([1, 4], F32)

            g = nc.gpsimd
            for tile, dram in [(idn, idn_d), (fw1d, fw1d_d), (fw1e, fw1e_d),
                               (fb1, fb1_d), (fw2, fw2_d), (hw5d, hw5d_d),
                               (hw5e, hw5e_d), (csrow, csb_d), (bhrow, bhb_d)]:
                g.dma_start(tile[:], dram[:])
            nc.vector.memset(SS[:], 0.0)
            nc.vector.memset(Zro[:], 0.0)
            nc.vector.memset(Zro2[:], 0.0)
            nc.vector.memset(HDt[:], 0.0)
            nc.vector.memset(ones1[:], 1.0)

            # ================= SCAN =================
            with tc.tile_pool(name="scons", bufs=1) as sp, \
                 tc.tile_pool(name="work", bufs=1) as wp, \
                 tc.tile_pool(name="ps_scan", bufs=1, space="PSUM") as pp:
                X_sb = sp.tile([128, 2 * TF], F32)      # [b128, (chunk, t, f)]
                xe0 = sp.tile([128, F + 2], F32)        # [x_in | ones | y] chunk0
                xe1 = sp.tile([128, F + 2], F32)
                xT = sp.tile([F + 2, BC], F32)          # transposed staging
                ewhh = sp.tile([HE, G4], F32)
                ewihb = sp.tile([F + 1, G4], F32)
                aw1h = sp.tile([HE, HE], F32)
                aw1c = sp.tile([HE, HE], F32)
                ab1 = sp.tile([HE, 1], F32)
                aw2 = sp.tile([HE, F], F32)
                ab2 = sp.tile([1, F], F32)
                dwhh = sp.tile([HD, G4], F32)
                dwihb = sp.tile([34, G4], F32)          # rows 32..33 hold [bias; wih]

                for tile, dram in [(ewhh, ewhh_d), (ewihb, ewihb_d), (aw1h, aw1h_d),
                                   (aw1c, aw1c_d), (ab1, ab1_d), (aw2, aw2_d),
                                   (ab2, ab2_d), (dwhh, dwhh_d)]:
                    g.dma_start(tile[:], dram[:])
                g.dma_start(dwihb[32:34, :], dwihb_d[:])
                nslab = max(1, T // 16)
                slab = TF // nslab
                for ch in range(2):
                    for s in range(nslab):
                        nc.sync.dma_start(
                            X_sb[:, ch * TF + s * slab: ch * TF + (s + 1) * slab],
                            Xd[ch * 128:(ch + 1) * 128, s * slab:(s + 1) * slab])
                nc.vector.memset(xe0[:, F:F + 1], 1.0)
                nc.vector.memset(xe1[:, F:F + 1], 1.0)
                xech = (xe0, xe1)

                for t in range(T):
                    dec = t < T - 1
                    W = 2 * BC if dec else BC
                    HDp = Zro2[:] if t == 0 else HDt[:]
                    Hp = Zro[:] if t == 0 else HDt[:, 0:BC]

                    gps = pp.tile([128, 4 * 2 * BC], F32, name="gps", tag="gps")
                    # Opening MM writes the WHOLE bank (one accumulation group per
                    # bank -- start=True zeroes the full 2KB bank).  Dec half gets
                    # ewhh.T @ D, fixed up by the (dwhh-ewhh) correction MM below.
                    for gi in range(4):
                        nc.tensor.matmul(gps[:, gi * 2 * BC:(gi + 1) * 2 * BC],
                                         ewhh[:, gi * HE:(gi + 1) * HE], HDp,
                                         start=True, stop=False)
                    if dec:
                        for gi in range(4):
                            reg = gps[:, gi * 2 * BC + BC: (gi + 1) * 2 * BC]
                            nc.tensor.matmul(reg, dwhh[:, gi * HE:(gi + 1) * HE],
                                             HDt[:, BC:2 * BC] if t else Zro[:],
                                             start=False, stop=False)

                    # ---- input attention ----
                    aps = pp.tile([128, BC], F32, name="aps", tag="aps")
                    nc.tensor.matmul(aps[:], aw1h[:], Hp, start=True, stop=False)
                    nc.tensor.matmul(aps[:], aw1c[:], SS[:, 0:BC], start=False,
                                     stop=True)
                    tanh_at = wp.tile([128, BC], F32, name="tanh_at", tag="tanh_at",
                                      bufs=2)
                    nc.scalar.activation(tanh_at[:], aps[:], AF.Tanh, bias=ab1[:])

                    scps = pp.tile([128, 2 * F], F32, name="scps", tag="scps")
                    for ch in range(2):
                        nc.tensor.matmul(scps[:, ch * F:(ch + 1) * F],
                                         tanh_at[:, ch * 128:(ch + 1) * 128], aw2[:],
                                         start=True, stop=not b2_nonzero)
                        if b2_nonzero:
                            nc.tensor.matmul(scps[:, ch * F:(ch + 1) * F],
                                             ones1[:], ab2[:], start=False, stop=True)
                    expt = wp.tile([128, 2 * F], F32, name="expt", tag="expt", bufs=2)
                    sums = wp.tile([128, 2], F32, name="sums", tag="sums", bufs=2)
                    for ch in range(2):
                        nc.scalar.activation(expt[:, ch * F:(ch + 1) * F],
                                             scps[:, ch * F:(ch + 1) * F], AF.Exp,
                                             accum_out=sums[:, ch:ch + 1])
                    rcp = wp.tile([128, 2], F32, name="rcp", tag="rcp", bufs=2)
                    nc.vector.reciprocal(rcp[:], sums[:])
                    for ch in range(2):
                        # x_in = (exp * 1/sum) * x_t
                        nc.vector.scalar_tensor_tensor(
                            xech[ch][:, 0:F], expt[:, ch * F:(ch + 1) * F],
                            rcp[:, ch:ch + 1],
                            X_sb[:, ch * TF + t * F: ch * TF + t * F + F],
                            OP.mult, OP.mult)
                        if dec:
                            nc.vector.tensor_copy(
                                xech[ch][:, F + 1:F + 2],
                                X_sb[:, ch * TF + t * F + close_idx:
                                     ch * TF + t * F + close_idx + 1])
                    xtps = pp.tile([F + 2, 2 * 128], F32, name="xtps", tag="xtps")
                    for ch in range(2):
                        nc.tensor.transpose(xtps[:, ch * 128:(ch + 1) * 128],
                                            xech[ch][:, 0:F + 2], idn[:])
                    nc.any.tensor_copy(xT[:], xtps[:])
                    if debug and t == 1:
                        nc.sync.dma_start(dbg_xT[:], xT[:])

                    # ---- input-side gate matmuls ----
                    for gi in range(4):
                        reg = gps[:, gi * 2 * BC: gi * 2 * BC + BC]
                        nc.tensor.matmul(reg, ewihb[:, gi * HE:(gi + 1) * HE],
                                         xT[0:F + 1, :], start=False, stop=not dec)
                    if dec:
                        for gi in range(4):
                            reg = gps[:, gi * 2 * BC + BC: (gi + 1) * 2 * BC]
                            nc.tensor.matmul(reg,
                                             dwihb[32:34, gi * HE:(gi + 1) * HE],
                                             xT[F:F + 2, :], start=False, stop=True)

                    # ---- gate tanh + cell ----
                    th = wp.tile([128, 4 * 2 * BC], F32, name="th", tag="th")
                    if dec:
                        nc.scalar.activation(th[:], gps[:], AF.Tanh)
                    else:
                        thv = th[:].rearrange("p (g w) -> p g w", g=4)[:, :, 0:BC]
                        gpv = gps[:].rearrange("p (g w) -> p g w", g=4)[:, :, 0:BC]
                        nc.scalar.activation(thv, gpv, AF.Tanh)

                    if debug and t == 1:
                        nc.sync.dma_start(dbg_th[:], th[:])
                        nc.sync.dma_start(dbg_t0[:], tanh_at[:])

                    def gsl(gi):
                        return th[:, gi * 2 * BC: gi * 2 * BC + W]

                    u = wp.tile([128, 2 * BC], F32, name="u", tag="u")
                    v = wp.tile([128, 2 * BC], F32, name="v", tag="v")
                    # S_new = 0.5*(1+tf)*S + (1+ti)*tg   (S = 2c)
                    nc.vector.scalar_tensor_tensor(u[:, 0:W], gsl(1), 1.0, SS[:, 0:W],
                                                   OP.add, OP.mult)
                    nc.vector.scalar_tensor_tensor(v[:, 0:W], gsl(0), 1.0, gsl(2),
                                                   OP.add, OP.mult)
                    nc.vector.scalar_tensor_tensor(SS[:, 0:W], u[:, 0:W], 0.5,
                                                   v[:, 0:W], OP.mult, OP.add)
                    thc = wp.tile([128, 2 * BC], F32, name="thc", tag="thc")
                    nc.scalar.activation(thc[:, 0:W], SS[:, 0:W], AF.Tanh, scale=0.5)
                    # H_new = (1+to)*tanh(c)   (H = 2h)
                    nc.vector.scalar_tensor_tensor(HDt[:, 0:BC],
                                                   gsl(3)[:, 0:BC], 1.0, thc[:, 0:BC],
                                                   OP.add, OP.mult)
                    nc.any.tensor_copy(encT[:, t * BC:(t + 1) * BC], HDt[:, 0:BC])
                    if dec:
                        nc.vector.scalar_tensor_tensor(
                            HDt[:, BC:2 * BC], th[:, 3 * 2 * BC + BC: 4 * 2 * BC], 1.0,
                            thc[:, BC: 2 * BC], OP.add, OP.mult)

            if debug:
                nc.sync.dma_start(dbg_encT[:], encT[:])
                nc.sync.dma_start(dbg_D[:], HDt[:, BC:2 * BC])
                nc.sync.dma_start(dbg_SS[:], SS[:])
            # ================= FINAL =================
            with tc.tile_pool(name="fwork", bufs=1) as fw, \
                 tc.tile_pool(name="ps_fin", bufs=1, space="PSUM") as fp:
                ecomb = fw.tile([64, 32 * 128], F32, name="ecomb")
                g.dma_start(ecomb[:], ecomb_d[:])

                # temporal attention scores
                scf = fp.tile([128, 2 * T], F32, name="scf", tag="scf")
                for t in range(T):
                    pre = fp.tile([128, BC], F32, name="pre", tag="pre", bufs=2)
                    nc.tensor.matmul(pre[:], fw1d[:], HDt[:, BC:2 * BC], start=True, stop=False)
                    nc.tensor.matmul(pre[:], fw1e[:], encT[:, t * BC:(t + 1) * BC],
                                     start=False, stop=True)
                    tf_t = fw.tile([128, BC], F32, name="tf_t", tag="tf_t", bufs=4)
                    nc.scalar.activation(tf_t[:], pre[:], AF.Tanh, bias=fb1[:])
                    for ch in range(2):
                        nc.tensor.matmul(scf[:, ch * T + t: ch * T + t + 1],
                                         tf_t[:, ch * 128:(ch + 1) * 128], fw2[:],
                                         start=True, stop=True)
                # softmax over t (scaled by 0.5 to undo enc=2h)
                bexp = fw.tile([128, 2 * T], F32, name="bexp")
                bsum = fw.tile([128, 2], F32, name="bsum")
                for ch in range(2):
                    nc.scalar.activation(bexp[:, ch * T:(ch + 1) * T],
                                         scf[:, ch * T:(ch + 1) * T], AF.Exp,
                                         accum_out=bsum[:, ch:ch + 1])
                if debug:
                    nc.sync.dma_start(dbg_sc[:], bexp[:])
                brcp = fw.tile([128, 2], F32, name="brcp")
                nc.vector.reciprocal(brcp[:], bsum[:])
                brcph = fw.tile([128, 2], F32, name="brcph")
                nc.vector.tensor_scalar(brcph[:], brcp[:], 0.5, None, OP.mult)
                beta = fw.tile([128, 2 * T], F32, name="beta")
                for ch in range(2):
                    nc.vector.tensor_scalar(beta[:, ch * T:(ch + 1) * T],
                                            bexp[:, ch * T:(ch + 1) * T],
                                            brcph[:, ch:ch + 1], None, OP.mult)
                # transpose beta -> beta_T halves [64, 2*128] at base 0
                nhalf = (T + 63) // 64
                bth = []
                for h in range(nhalf):
                    hw = min(64, T - h * 64)
                    btps = fp.tile([64, 2 * 128], F32, name="btps", tag="small")
                    for ch in range(2):
                        nc.tensor.transpose(
                            btps[0:hw, ch * 128:(ch + 1) * 128],
                            beta[:, ch * T + h * 64: ch * T + h * 64 + hw], idn[:])
                    bt_h = fw.tile([64, BC], F32, name="bt_h", tag=f"bt_{h}")
                    nc.any.tensor_copy(bt_h[0:hw, :], btps[0:hw, :])
                    bth.append(bt_h)

                if debug:
                    nc.sync.dma_start(dbg_beta[:], beta[:])
                # context: ctx_T[h, b] = sum_t encT[h, (t,b)] * beta'[b, t]
                GRP = 8
                ngrp = (T + GRP - 1) // GRP
                ctxT = fw.tile([128, BC], F32, name="ctxT")
                parts = fw.tile([128, ngrp * BC], F32, name="parts")
                for grp in range(ngrp):
                    n_in = min(GRP, T - grp * GRP)
                    bc_ps = fp.tile([128, GRP * BC], F32, name="bc_ps", tag="bc_ps")
                    for j in range(n_in):
                        t = grp * GRP + j
                        h, r = t // 64, t % 64
                        if r < 32:
                            lhs = ecomb[0:32, r * 128:(r + 1) * 128]
                            rhs = bth[h][0:32, :]
                        else:
                            lhs = ecomb[32:64, (r - 32) * 128:(r - 31) * 128]
                            rhs = bth[h][32:64, :]
                        nc.tensor.matmul(bc_ps[:, j * BC:(j + 1) * BC], lhs, rhs,
                                         start=True, stop=True)
                    prod = fw.tile([128, GRP * BC], F32, name="prod", tag="prod")
                    nc.vector.tensor_tensor(prod[:, 0:n_in * BC],
                                            encT[:, grp * GRP * BC:
                                                 (grp * GRP + n_in) * BC],
                                            bc_ps[:, 0:n_in * BC], OP.mult)
                    pv = prod[:, 0:n_in * BC].rearrange("p (t b) -> p b t", t=n_in)
                    nc.vector.tensor_reduce(parts[:, grp * BC:(grp + 1) * BC], pv,
                                            axis=AX.X, op=OP.add)
                pav = parts[:].rearrange("p (g b) -> p b g", g=ngrp)
                nc.vector.tensor_reduce(ctxT[:], pav, axis=AX.X, op=OP.add)

                if debug:
                    nc.sync.dma_start(dbg_ctx[:], ctxT[:])
                # cat_nat = [d | ctx] in natural layout (d = D/2)
                cat = fw.tile([128, 2 * (HD + HE)], F32, name="cat")
                for ch in range(2):
                    tp = fp.tile([128, 128], F32, name="tp", tag="small")
                    nc.tensor.transpose(tp[:], HDt[:, BC + ch * 128: BC + (ch + 1) * 128], idn[:])
                    nc.scalar.activation(cat[:, ch * 256: ch * 256 + 128], tp[:],
                                         AF.Copy, scale=0.5)
                    tp2 = fp.tile([128, 128], F32, name="tp2", tag="small")
                    nc.tensor.transpose(tp2[:], ctxT[:, ch * 128:(ch + 1) * 128],
                                        idn[:])
                    nc.scalar.activation(cat[:, ch * 256 + 128: ch * 256 + 256],
                                         tp2[:], AF.Copy, scale=1.0)

                if debug:
                    nc.sync.dma_start(dbg_cat[:], cat[:])
                # LayerNorm stats
                msum = fw.tile([128, 2], F32, name="msum")
                cv = cat[:].rearrange("p (c f) -> p c f", c=2)
                nc.vector.tensor_reduce(msum[:], cv, axis=AX.X, op=OP.add)
                mu = fw.tile([128, 2], F32, name="mu")
                nc.vector.tensor_scalar(mu[:], msum[:], 1.0 / 256.0, None, OP.mult)
                sqs = fw.tile([128, 2], F32, name="sqs")
                for ch in range(2):
                    sqtmp = fw.tile([128, 256], F32, name="sqtmp", tag="sqtmp",
                                    bufs=2)
                    nc.scalar.activation(sqtmp[:], cat[:, ch * 256:(ch + 1) * 256],
                                         AF.Square, accum_out=sqs[:, ch:ch + 1])
                q = fw.tile([128, 2], F32, name="q")
                nc.vector.tensor_scalar(q[:], sqs[:], 1.0 / 256.0, None, OP.mult)
                mm2 = fw.tile([128, 2], F32, name="mm2")
                for ch in range(2):
                    nc.vector.tensor_scalar(mm2[:, ch:ch + 1], mu[:, ch:ch + 1],
                                            mu[:, ch:ch + 1], None, OP.mult)
                var = fw.tile([128, 2], F32, name="var")
                nc.vector.tensor_tensor(var[:], q[:], mm2[:], OP.subtract)
                epst = fw.tile([128, 1], F32, name="epst")
                nc.vector.memset(epst[:], 1e-5)
                std = fw.tile([128, 2], F32, name="std")
                nc.scalar.activation(std[:], var[:], AF.Sqrt, bias=epst[:])
                rstd = fw.tile([128, 2], F32, name="rstd")
                nc.vector.reciprocal(rstd[:], std[:])

                # heads: raw_T = W'.T @ cat_T  (gamma/beta folded on host)
                raw_ps = fp.tile([4, BC], F32, name="raw_ps", tag="small")
                nc.tensor.matmul(raw_ps[:], hw5d[:], HDt[:, BC:2 * BC], start=True, stop=False)
                nc.tensor.matmul(raw_ps[:], hw5e[:], ctxT[:], start=False, stop=True)
                rawT = fw.tile([4, BC], F32, name="rawT")
                nc.any.tensor_copy(rawT[:], raw_ps[:])
                raw_nat = fw.tile([128, 8], F32, name="raw_nat")
                for ch in range(2):
                    rp = fp.tile([128, 4], F32, name="rp", tag="small")
                    nc.tensor.transpose(rp[:], rawT[:, ch * 128:(ch + 1) * 128],
                                        idn[0:4, 0:4])
                    nc.any.tensor_copy(raw_nat[:, ch * 4:(ch + 1) * 4], rp[:])
                # broadcasts of cs and bias rows
                cb_ps = fp.tile([128, 8], F32, name="cb_ps", tag="small")
                nc.tensor.matmul(cb_ps[:, 0:4], ones1[:], csrow[:], start=True,
                                 stop=True)
                nc.tensor.matmul(cb_ps[:, 4:8], ones1[:], bhrow[:], start=True,
                                 stop=True)
                csb = fw.tile([128, 8], F32, name="csb")
                nc.any.tensor_copy(csb[:], cb_ps[:])

                out_sb = fw.tile([128, 8], F32, name="out_sb")
                m2 = fw.tile([128, 2], F32, name="m2")
                for ch in range(2):
                    nc.vector.tensor_scalar(m2[:, ch:ch + 1], mu[:, ch:ch + 1],
                                            rstd[:, ch:ch + 1], -1.0, OP.mult,
                                            OP.mult)
                o1 = fw.tile([128, 8], F32, name="o1")
                for ch in range(2):
                    nc.vector.tensor_scalar(o1[:, ch * 4:(ch + 1) * 4],
                                            raw_nat[:, ch * 4:(ch + 1) * 4],
                                            rstd[:, ch:ch + 1], None, OP.mult)
                    nc.vector.scalar_tensor_tensor(out_sb[:, ch * 4:(ch + 1) * 4],
                                                   csb[:, 0:4], m2[:, ch:ch + 1],
                                                   o1[:, ch * 4:(ch + 1) * 4],
                                                   OP.mult, OP.add)
                nc.vector.tensor_tensor(out_sb[:, 0:4], out_sb[:, 0:4], csb[:, 4:8],
                                        OP.add)
                nc.vector.tensor_tensor(out_sb[:, 4:8], out_sb[:, 4:8], csb[:, 4:8],
                                        OP.add)
                for ch in range(2):
                    nc.sync.dma_start(outd[ch * 128:(ch + 1) * 128, :],
                                      out_sb[:, ch * 4:(ch + 1) * 4])

    nc.finalize()
    return nc


def _prep_weights(inputs):
    """Host-side weight folding. Returns dict of per-core-replicated arrays."""
    f32 = np.float32
    gs = np.concatenate([np.full(HE, 0.5), np.full(HE, 0.5),
                         np.full(HE, 1.0), np.full(HE, 0.5)]).astype(f32)  # i,f,g,o

    enc_Wih = np.asarray(inputs["enc_Wih"], f32)
    enc_Whh = np.asarray(inputs["enc_Whh"], f32)
    enc_b = np.asarray(inputs["enc_b"], f32)
    dec_Wih = np.asarray(inputs["dec_Wih"], f32)
    dec_Whh = np.asarray(inputs["dec_Whh"], f32)
    dec_b = np.asarray(inputs["dec_b"], f32)

    w = {}
    w["ewhh"] = np.ascontiguousarray(enc_Whh * (gs * 0.5)[None, :])
    w["ewihb"] = np.ascontiguousarray(
        np.vstack([enc_Wih * gs[None, :], (enc_b * gs)[None, :]]))
    # correction weights: opening gate MM applies ewhh to the dec half too
    w["dwhh"] = np.ascontiguousarray(
        dec_Whh * (gs * 0.5)[None, :] - enc_Whh * (gs * 0.5)[None, :])
    w["dwihb"] = np.ascontiguousarray(
        np.vstack([(dec_b * gs)[None, :], dec_Wih[0:1, :] * gs[None, :]]))

    aw1 = np.asarray(inputs["enc_a_w1"], f32)
    w["aw1h"] = np.ascontiguousarray(0.5 * aw1[0:HE])
    w["aw1c"] = np.ascontiguousarray(0.5 * aw1[HE:2 * HE])
    w["ab1"] = np.ascontiguousarray(np.asarray(inputs["enc_a_b1"], f32).reshape(HE, 1))
    w["aw2"] = np.ascontiguousarray(np.asarray(inputs["enc_a_w2"], f32))
    w["ab2"] = np.ascontiguousarray(np.asarray(inputs["enc_a_b2"], f32).reshape(1, F))

    dw1 = np.asarray(inputs["dec_a_w1"], f32)
    w["fw1d"] = np.ascontiguousarray(0.5 * dw1[0:HD])
    w["fw1e"] = np.ascontiguousarray(0.5 * dw1[HD:HD + HE])
    w["fb1"] = np.ascontiguousarray(np.asarray(inputs["dec_a_b1"], f32).reshape(HE, 1))
    w["fw2"] = np.ascontiguousarray(np.asarray(inputs["dec_a_w2"], f32).reshape(HE, 1))

    gamma = np.asarray(inputs["ln_gamma"], f32)
    lbeta = np.asarray(inputs["ln_beta"], f32)
    Wh = np.hstack([np.asarray(inputs["reg_w"], f32),
                    np.asarray(inputs["cls_w"], f32)])          # [256, 4]
    bh = np.concatenate([np.asarray(inputs["reg_b"], f32).ravel(),
                         np.asarray(inputs["cls_b"], f32).ravel()])  # [4]
    Wp = Wh * gamma[:, None]
    w["hw5d"] = np.ascontiguousarray(0.5 * Wp[0:HD])
    w["hw5e"] = np.ascontiguousarray(Wp[HD:HD + HE])
    w["csrow"] = np.ascontiguousarray(Wp.sum(0).reshape(1, 4))
    w["bhrow"] = np.ascontiguousarray((bh + lbeta @ Wh).reshape(1, 4))

    w["idn"] = np.eye(128, dtype=f32)
    ecomb = np.zeros((64, 32 * 128), f32)
    for j in range(32):
        ecomb[j, j * 128:(j + 1) * 128] = 1.0
        ecomb[32 + j, j * 128:(j + 1) * 128] = 1.0
    w["ecomb"] = ecomb
    return w


def kernel(**inputs):
    from concourse.bass_utils import run_bass_kernel_spmd

    X = np.asarray(inputs["X"], np.float32)
    Bv, T, Fv = X.shape
    assert Bv == B and Fv == F
    close_idx = int(np.asarray(inputs["close_pct_idx"]))
    w = _prep_weights(inputs)
    b2_nonzero = bool(np.any(w["ab2"]))

    key = (T, close_idx, b2_nonzero)
    if key not in _cache:
        _cache[key] = _build(T, close_idx, b2_nonzero)
    nc = _cache[key]

    in_maps = []
    for c in range(NCORES):
        m = dict(w)
        m["x"] = np.ascontiguousarray(X[c * BC:(c + 1) * BC].reshape(BC, T * F))
        in_maps.append(m)

    res = run_bass_kernel_spmd(nc, in_maps, core_ids=list(range(NCORES)))
    full = np.concatenate([res.results[c]["out"] for c in range(NCORES)], axis=0)
    return (np.ascontiguousarray(full[:, 0:1]),
            np.ascontiguousarray(full[:, 1:4]))
